# revision 8
# baseline (speedup 1.0000x reference)
"""Binary-weight 3x3 SAME conv + bias + ReLU for (16,224,224,64)x(3,3,64,128),
distributed over 8 Trainium2 NeuronCores.

Distribution: data-parallel over the batch (N=16 -> 2 images per core), conv
weights (tiny, binarized to +/-1) replicated on every core. Forward only, so
no collectives are needed.

Per-core kernel design (fp8 DoubleRow, hybrid-precision):
  - The PE's fp8e4 DoubleRow perf mode contracts TWO 64-partition k-tiles per
    matmul at the same 450-row cadence as one fp16 matmul: 2x throughput.
    Inputs are quantized to fp8_e4m3 (x8) with a residual channel
    r8 = e4m3(x - x8). 5 of the 9 conv taps are residual-corrected, which
    keeps the scale-relative absmax error at ~1.7e-2 (host-validated on the
    fixed problem seed; gate is 2e-2); correcting all 9 would cost exactly
    the fp16 time, correcting none measures 2.5e-2.
  - SBUF strip layout [128p, 3, rows*wp] fp8: partitions 0-63 image 0,
    64-127 image 1; free regions (r8, x8, x8 shifted one row). Every
    DoubleRow matmul takes its two k-tiles as a natural dim1 slice:
    regions [0:2] at one tap offset = (residual, main) of a corrected tap;
    regions [1:3] = main taps (dy,dx)+(dy+1,dx) vertically paired. 9 main
    taps + 5 residual taps pack into exactly 7 matmuls per image-duo
    (vs 9 fp16 slots). The two images' matmuls land on disjoint PE
    row-groups (tile_position (0,0)/(64,0)) and run concurrently -
    measured 199.5ns per pair-slot on hardware.
  - A PSUM tile covers two padded output rows (450 of 512 fp32); pad
    columns compute garbage that is never drained. Drain = bias-add + ReLU
    out of PSUM alternating ScalarE/VectorE into fp16 staging tiles, which
    halves the output DMA bytes (25.7MB/core); host upcasts to fp32.
  - Quantization error budget: weights are exact +/-1 in fp8; accumulation
    is fp32 PSUM; fp16 staging adds ~5e-4 rel; the dominant term is the 4
    uncorrected taps' e4m3 input rounding.
"""

import os
import sys

import numpy as np
import ml_dtypes

for _p in ("/opt/trn_rl_repo", "/root/.axon_site/_ro/trn_rl_repo", "/root/.axon_site"):
    if os.path.isdir(_p) and _p not in sys.path:
        sys.path.append(_p)

import concourse.bass as bass
import concourse.mybir as mybir
import concourse.tile as tile
from concourse import bacc
from concourse.bass_utils import run_bass_kernel_spmd

# Problem shape (hardcoded per contract).
N_FULL, H, W_, CIN, COUT = 16, 224, 224, 64, 128
N_CORES = 8
IMGS = N_FULL // N_CORES  # images per core

# Tap schedule: 7 DoubleRow matmuls per image-duo. Each entry is
# (region_start, (dy, dx), (wtap0, wtap1)): k-tile j reads strip region
# region_start+j at flattened offset (2d+dy)*wp+dx, with weights of conv
# tap wtapj. Regions: 0 = r8, 1 = x8, 2 = x8 shifted one row down.
CORRECTED = [(0, 0), (0, 1), (1, 0), (2, 0), (2, 2)]
MM_SCHED = [(0, t, (t, t)) for t in CORRECTED] + [
    (1, (0, 2), ((0, 2), (1, 2))),
    (1, (1, 1), ((1, 1), (2, 1))),
]


def build_conv_program(imgs=IMGS, h=H, w=W_, cin=CIN, cout=COUT, strip_out=16):
    assert imgs == 2 and cin == 64 and cout == 128
    assert h % strip_out == 0 and strip_out % 2 == 0
    hp, wp = h + 2, w + 2
    n_strips = h // strip_out
    pairs = strip_out // 2  # output-row pairs per strip
    rows = strip_out + 2  # input rows needed per strip
    F = rows * wp  # strip region size (flattened rows)
    nfree = 2 * wp - 2  # matmul free dim (2 padded rows, minus trailing pads)
    npsum = 2 * wp  # PSUM tile width (trailing 2 cols never written/read)
    assert nfree <= 512
    f8, f16, f32 = mybir.dt.float8e4, mybir.dt.float16, mybir.dt.float32
    n_mm = len(MM_SCHED)

    nc = bacc.Bacc("TRN2", target_bir_lowering=False, debug=False)
    xr8 = nc.dram_tensor("xr8", [imgs, cin, 3, hp * wp], f8, kind="ExternalInput")
    wq = nc.dram_tensor("wq", [2 * cin, n_mm, 2, cout], f8, kind="ExternalInput")
    bias = nc.dram_tensor("bias", [cout, 1], f32, kind="ExternalInput")
    out = nc.dram_tensor("out", [imgs, cout, h, w], f16, kind="ExternalOutput")

    with tile.TileContext(nc) as tc:
        with (
            tc.tile_pool(name="const", bufs=1) as cpool,
            tc.tile_pool(name="xin", bufs=2) as xpool,
            tc.tile_pool(name="ps", bufs=8, space="PSUM") as pspool,
            tc.tile_pool(name="ostage", bufs=4) as opool,
        ):
            wsb = cpool.tile([128, n_mm, 2, cout], f8)
            nc.sync.dma_start(out=wsb[:], in_=wq[:])
            bsb = cpool.tile([cout, 1], f32)
            nc.sync.dma_start(out=bsb[:], in_=bias[:])
            warm = cpool.tile([cout, 1], f32)

            for s in range(n_strips):
                r0 = s * strip_out
                # Strip tile: both images' channels stacked on partitions;
                # 3 fp8 regions (r8 | x8 | x8 shifted one row) per image.
                xt = xpool.tile([128, 3, F], f8)
                # Strip loads on the scalar engine's HWDGE queue (away from
                # bulk output traffic on sync); one 3D descriptor per chunk
                # covers all three regions. The first strip is split in four
                # chunks spread over four queues so the PE unblocks after
                # ~1/4 of the load instead of half.
                if s == 0:
                    chunks = [(0, 6), (6, 10), (10, 14), (14, rows)]
                    engines = [nc.scalar, nc.sync, nc.gpsimd, nc.scalar]
                else:
                    rh0 = rows // 2
                    chunks = [(0, rh0), (rh0, rows)]
                    engines = [nc.scalar, nc.scalar]
                for (ra, rb), eng in zip(chunks, engines):
                    eng.dma_start(
                        out=xt[:, :, ra * wp : rb * wp],
                        in_=xr8[:][
                            :, :, :, (r0 + ra) * wp : (r0 + rb) * wp
                        ].rearrange("i c s f -> (i c) s f"),
                    )
                if s == 0:
                    # Warm the ACT Relu spline table (~2.7us one-time load,
                    # unmodeled by the scheduler) AFTER the first strip's
                    # load triggers.
                    nc.scalar.activation(
                        warm[:], bsb[:], mybir.ActivationFunctionType.Relu, bias=0.0
                    )

                ots = [
                    opool.tile([cout, strip_out * w], f16, name=f"ot{i}", tag=f"ot{i}")
                    for i in range(imgs)
                ]

                for d in range(pairs):
                    pt = [
                        pspool.tile([cout, npsum], f32, name=f"pt_{s}_{d}_{i}", tag="pt")
                        for i in range(imgs)
                    ]
                    # 7 DoubleRow matmuls accumulate 9 main + 5 residual
                    # taps; emission order (m, i) alternates images so
                    # adjacent PE matmuls land on disjoint row-groups and
                    # run concurrently.
                    for m, (sreg, (dy, dx), _wt) in enumerate(MM_SCHED):
                        base = (2 * d + dy) * wp + dx
                        for i in range(imgs):
                            nc.tensor.matmul(
                                pt[i][:, :nfree],
                                wsb[i * cin : (i + 1) * cin, m],
                                xt[
                                    i * cin : (i + 1) * cin,
                                    sreg : sreg + 2,
                                    base : base + nfree,
                                ],
                                start=(m == 0),
                                stop=(m == n_mm - 1),
                                perf_mode=mybir.MatmulPerfMode.DoubleRow,
                            )
                    # Drain: bias + ReLU, skipping the 2 pad columns per row.
                    for i in range(imgs):
                        src = pt[i][:].rearrange("p (r q) -> p r q", q=wp)[:, :, :w]
                        dst = ots[i][:, d * 2 * w : (d + 1) * 2 * w].rearrange(
                            "p (r q) -> p r q", q=w
                        )
                        if (d + i) % 2 == 0:
                            nc.scalar.activation(
                                dst,
                                src,
                                mybir.ActivationFunctionType.Relu,
                                bias=bsb[:, 0:1],
                            )
                        else:
                            nc.vector.tensor_scalar(
                                dst,
                                src,
                                bsb[:, 0:1],
                                0.0,
                                mybir.AluOpType.add,
                                mybir.AluOpType.max,
                            )
                    # Half-strip output DMA on the otherwise-idle sync
                    # engine; per-duo on the final strip so the kernel-tail
                    # transfer is short.
                    qg = 1 if s == n_strips - 1 else max(1, pairs // 2)
                    if d % qg == qg - 1:
                        quart = d // qg
                        rh = 2 * qg
                        for i in range(imgs):
                            nc.sync.dma_start(
                                out=out[:][
                                    i, :, r0 + quart * rh : r0 + (quart + 1) * rh, :
                                ].rearrange("c r q -> c (r q)"),
                                in_=ots[i][:, quart * rh * w : (quart + 1) * rh * w],
                            )

    nc.compile()
    return nc


def prep_inputs(x, W, b, imgs=IMGS, h=H, w=W_, cin=CIN, cout=COUT, n_cores=N_CORES):
    """Host-side shard + fp8 split + layout prep. Returns per-core input maps."""
    hp, wp = h + 2, w + 2
    n = imgs * n_cores
    f8 = ml_dtypes.float8_e4m3
    n_mm = len(MM_SCHED)

    # Binarize weights; pack per-matmul DoubleRow k-tile pairs, duplicated
    # per image slot.
    wq_np = np.sign(np.asarray(W, dtype=np.float32)).astype(f8)
    wq_host = np.empty((2 * cin, n_mm, 2, cout), f8)
    for m, (_sreg, _tap, (t0, t1)) in enumerate(MM_SCHED):
        for j, t in enumerate((t0, t1)):
            wq_host[0:cin, m, j] = wq_np[t[0], t[1]]
            wq_host[cin : 2 * cin, m, j] = wq_np[t[0], t[1]]
    bias_host = np.ascontiguousarray(np.asarray(b, dtype=np.float32).reshape(cout, 1))

    # fp8 split: x8 = e4m3(x), r8 = e4m3(x - x8), NHWC -> NCHW, 1-px halo.
    xf = np.asarray(x, dtype=np.float32)
    x8 = xf.astype(f8)
    r8 = (xf - x8.astype(np.float32)).astype(f8)
    xr8_host = np.zeros((n, cin, 3, hp, wp), f8)
    xr8_host[:, :, 0, 1 : h + 1, 1 : w + 1] = r8.transpose(0, 3, 1, 2)
    xr8_host[:, :, 1, 1 : h + 1, 1 : w + 1] = x8.transpose(0, 3, 1, 2)
    flat = xr8_host.reshape(n, cin, 3, hp * wp)
    # Region 2 = region 1 shifted one padded row down (exact: the conv
    # never reads past the bottom halo through this region).
    flat[:, :, 2, : (hp - 1) * wp] = flat[:, :, 1, wp:]
    return [
        {
            "xr8": np.ascontiguousarray(flat[c * imgs : (c + 1) * imgs]),
            "wq": wq_host,
            "bias": bias_host,
        }
        for c in range(n_cores)
    ]


_NC_CACHE = {}


def _get_program():
    if "nc" not in _NC_CACHE:
        _NC_CACHE["nc"] = build_conv_program()
    return _NC_CACHE["nc"]


def kernel(x, W, b):
    """Full-input entry point: x (16,224,224,64) f32 NHWC, W (3,3,64,128) f32
    HWIO, b (128,) f32 -> (16,224,224,128) f32 NHWC."""
    nc = _get_program()
    in_maps = prep_inputs(x, W, b)
    res = run_bass_kernel_spmd(nc, in_maps, core_ids=list(range(N_CORES)))
    # Gather: per-core [2, 128, 224, 224] f16 -> full NHWC f32.
    full = np.empty((N_FULL, H, W_, COUT), np.float32)
    for c in range(N_CORES):
        o = np.asarray(res.results[c]["out"]).astype(np.float32)
        full[c * IMGS : (c + 1) * IMGS] = o.transpose(0, 2, 3, 1)
    return full
